# revision 1
# baseline (speedup 1.0000x reference)
"""Bidirectional 2-layer LSTM on 8 trn2 NeuronCores, data-parallel over batch.

Math (per reference.py, flax LSTMCell): gates [i,f,g,o], input dense no bias,
hidden dense bias b (zero in this problem); c' = sigmoid(f)c + sigmoid(i)tanh(g);
h' = sigmoid(o)tanh(c').

Kernel reformulation (all rescalings are exact powers of two):
  sigmoid(z) = (1 + tanh(z/2))/2, so with host-prescaled weights every gate
  nonlinearity is a single tanh. States are doubled: C = 2c, Hs = 2h.
  For gates i,f,o the kernel computes u = z/2 (weights x0.5, Wh x0.25 since
  h = Hs/2, bias x0.5); for g it computes u = z (Wh x0.5).
    th_* = tanh(u_*)
    C' = 0.5*(th_f+1)*C + (th_i+1)*th_g          == 2c'
    Hs' = (th_o+1)*tanh(0.5*C')                  == 2h'
  Host divides outputs by 2 at the end.

Layout: gate-major. Hidden (128) on partitions, batch (B=4 per core) on the
free axis. PSUM bank per direction holds one 32-timestep chunk of gate
pre-activations: col = gate*128 + slot*4 + b. Input projections (one 128x128
matmul per gate per chunk per k-tile) accumulate into the bank ahead of time;
the per-step recurrent matmuls (128x128 stationary Wh gate tile, moving Hs
(128,4)) add on top with start=False.
"""
import os
import numpy as np

H = 128
B_CORE = 4
N_CORES = 8
CH = 32  # timesteps per psum-bank chunk: 4 gates * 32 * 4 batch = 512 fp32

# our gate order [i, f, o, g] as block indices into the reference's [i, f, g, o]
_GATE_PERM = (0, 1, 3, 2)

last_exec_time_ns = None  # set when LSTM_TRACE=1


def _prep_wx(w, extra=1.0):
    """(F, 512) reference layout -> our gate order + tanh-trick scaling."""
    w = np.asarray(w, np.float64)
    blocks = [w[:, i * H:(i + 1) * H] for i in _GATE_PERM]
    scales = [0.5 * extra, 0.5 * extra, 0.5 * extra, 1.0 * extra]
    return np.concatenate(
        [b * s for b, s in zip(blocks, scales)], axis=1
    ).astype(np.float16)


def _prep_wh(w):
    w = np.asarray(w, np.float64)
    blocks = [w[:, i * H:(i + 1) * H] for i in _GATE_PERM]
    scales = [0.25, 0.25, 0.25, 0.5]
    return np.concatenate(
        [b * s for b, s in zip(blocks, scales)], axis=1
    ).astype(np.float16)


_W_NAMES = ["wx0_f", "wx0_b", "wh0_f", "wh0_b", "wh1_f", "wh1_b",
            "wx1a_f", "wx1b_f", "wx1a_b", "wx1b_b"]


def build_nc(T):
    import concourse.bass as bass
    import concourse.mybir as mybir
    import concourse.tile as tile
    from contextlib import ExitStack

    fp32 = mybir.dt.float32
    fp16 = mybir.dt.float16
    TANH = mybir.ActivationFunctionType.Tanh
    ADD = mybir.AluOpType.add
    MULT = mybir.AluOpType.mult

    assert T % CH == 0
    NCH = T // CH

    nc = bass.Bass()

    dp = {}
    for nm in ("xT_f", "xT_b"):
        dp[nm] = nc.declare_dram_parameter(nm, [128, 4 * T], fp16, isOutput=False)
    for nm in _W_NAMES:
        dp[nm] = nc.declare_dram_parameter(nm, [128, 512], fp16, isOutput=False)
    out_d = {
        "f": nc.declare_dram_parameter("out_f", [128, 4 * T], fp32, isOutput=True),
        "b": nc.declare_dram_parameter("out_b", [128, 4 * T], fp32, isOutput=True),
    }
    cfin_d = {
        "f": nc.declare_dram_parameter("cfin_f", [128, 4], fp32, isOutput=True),
        "b": nc.declare_dram_parameter("cfin_b", [128, 4], fp32, isOutput=True),
    }

    with ExitStack() as ctx:
        tc = ctx.enter_context(tile.TileContext(nc))
        wpool = ctx.enter_context(tc.tile_pool(name="w", bufs=1))
        hpool = ctx.enter_context(tc.tile_pool(name="hbuf", bufs=1))
        psum = ctx.enter_context(tc.tile_pool(name="psum", bufs=1, space="PSUM"))
        xring = ctx.enter_context(tc.tile_pool(name="xring", bufs=4))
        small = ctx.enter_context(tc.tile_pool(name="small", bufs=4))
        state = ctx.enter_context(tc.tile_pool(name="state", bufs=1))

        w = {}
        for nm in _W_NAMES:
            wt = wpool.tile([128, 512], fp16, tag=nm, name=nm)
            nc.sync.dma_start(out=wt[:, :], in_=dp[nm][:, :])
            w[nm] = wt

        # H-state history buffers, slot k holds Hs_{k-1}; slot 0 = zeros
        hb = {(l, d): hpool.tile([128, 4 * (T + 1)], fp16, tag=f"h{l}{d}", name=f"h{l}{d}")
              for l in (0, 1) for d in "fb"}
        hb32 = {d: hpool.tile([128, 4 * T], fp32, tag=f"h32{d}", name=f"h32{d}")
                for d in "fb"}
        for buf in hb.values():
            nc.vector.memset(buf[:, 0:4], 0.0)

        cst = {(l, d): state.tile([128, 4], fp32, tag=f"c{l}{d}", name=f"c{l}{d}")
               for l in (0, 1) for d in "fb"}
        for t_ in cst.values():
            nc.vector.memset(t_[:, :], 0.0)

        zb = {d: [psum.tile([128, 512], fp32, tag=f"z{d}{k}", name=f"z{d}{k}") for k in (0, 1)]
              for d in "fb"}

        xtiles = {"f": {}, "b": {}}

        def emit_xdma(d, c):
            xt = xring.tile([128, 128], fp16, tag=f"x{d}", name=f"x{d}")
            nc.sync.dma_start(out=xt[:, :], in_=dp[f"xT_{d}"][:, c * 128:(c + 1) * 128])
            xtiles[d][c] = xt

        def emit_proj(layer, d, c):
            bank = zb[d][c % 2]
            if layer == 0:
                movs = [xtiles[d].pop(c)[:, :]]
                wxs = [w[f"wx0_{d}"]]
            else:
                rf = hb[(0, "f")][:, :].rearrange("p (s b) -> p s b", b=4)
                rb = hb[(0, "b")][:, :].rearrange("p (s b) -> p s b", b=4)
                fwd = slice(c * CH + 1, c * CH + CH + 1)
                rev = slice(T - c * CH, T - c * CH - CH, -1)
                if d == "f":
                    movs = [rf[:, fwd, :], rb[:, rev, :]]
                else:
                    movs = [rf[:, rev, :], rb[:, fwd, :]]
                wxs = [w[f"wx1a_{d}"], w[f"wx1b_{d}"]]
            for g in range(4):
                for ki, (mov, wx) in enumerate(zip(movs, wxs)):
                    nc.tensor.matmul(
                        bank[:, g * 128:(g + 1) * 128],
                        lhsT=wx[:, g * 128:(g + 1) * 128],
                        rhs=mov,
                        start=(g == 0 and ki == 0),
                        stop=False,
                    )

        def emit_slot(layer, d, c, s):
            t = c * CH + s
            bank = zb[d][c % 2]
            hbuf = hb[(layer, d)]
            whd = w[f"wh{layer}_{d}"]
            rhs = hbuf[:, t * 4:(t + 1) * 4]
            for g in range(4):
                nc.tensor.matmul(
                    bank[:, g * 128 + s * 4: g * 128 + (s + 1) * 4],
                    lhsT=whd[:, g * 128:(g + 1) * 128],
                    rhs=rhs,
                    start=False,
                    stop=(g == 3),
                )
            th = small.tile([128, 16], fp32, tag=f"th{d}", name=f"th{d}")
            zview = bank[:, :].rearrange("p (g s b) -> p g s b", s=CH, b=4)[:, :, s, :]
            nc.scalar.activation(
                th[:, :].rearrange("p (g b) -> p g b", b=4), zview, TANH)
            C = cst[(layer, d)]
            a = small.tile([128, 4], fp32, tag=f"a{d}", name=f"a{d}")
            b2 = small.tile([128, 4], fp32, tag=f"b{d}", name=f"b{d}")
            # b2 = (th_i + 1) * th_g
            nc.vector.scalar_tensor_tensor(b2[:, :], th[:, 0:4], 1.0, th[:, 12:16],
                                           op0=ADD, op1=MULT)
            # a = (th_f + 1) * C
            nc.vector.scalar_tensor_tensor(a[:, :], th[:, 4:8], 1.0, C[:, :],
                                           op0=ADD, op1=MULT)
            # C' = 0.5*a + b2
            nc.vector.scalar_tensor_tensor(C[:, :], a[:, :], 0.5, b2[:, :],
                                           op0=MULT, op1=ADD)
            thc = small.tile([128, 4], fp32, tag=f"thc{d}", name=f"thc{d}")
            nc.scalar.activation(thc[:, :], C[:, :], TANH, scale=0.5)
            # Hs' = (th_o + 1) * thc  -> fp16 into the history buffer (rec rhs)
            nc.vector.scalar_tensor_tensor(hbuf[:, (t + 1) * 4:(t + 2) * 4],
                                           th[:, 8:12], 1.0, thc[:, :],
                                           op0=ADD, op1=MULT)
            if layer == 1:
                # fp32 copy for the output (off the critical path)
                nc.vector.scalar_tensor_tensor(hb32[d][:, t * 4:(t + 1) * 4],
                                               th[:, 8:12], 1.0, thc[:, :],
                                               op0=ADD, op1=MULT)

        for layer in (0, 1):
            if layer == 0:
                for d in "fb":
                    emit_xdma(d, 0)
                    emit_xdma(d, 1)
            for d in "fb":
                emit_proj(layer, d, 0)
            for c in range(NCH):
                for s in range(CH):
                    if s == 4 and layer == 0 and c + 2 < NCH:
                        for d in "fb":
                            emit_xdma(d, c + 2)
                    if s == 8 and c + 1 < NCH:
                        for d in "fb":
                            emit_proj(layer, d, c + 1)
                    for d in "fb":
                        emit_slot(layer, d, c, s)
                if layer == 1:
                    for d in "fb":
                        nc.sync.dma_start(
                            out=out_d[d][:, c * 128:(c + 1) * 128],
                            in_=hb32[d][:, c * 128:(c + 1) * 128])
        for d in "fb":
            nc.sync.dma_start(out=cfin_d[d][:, :], in_=cst[(1, d)][:, :])

    _split_excess_waits(nc)
    return nc


def _prep_weight_maps(Wx_f0, Wh_f0, Wx_b0, Wh_b0, Wx_f1, Wh_f1, Wx_b1, Wh_b1):
    wm = {
        "wx0_f": _prep_wx(Wx_f0),
        "wx0_b": _prep_wx(Wx_b0),
        "wh0_f": _prep_wh(Wh_f0),
        "wh0_b": _prep_wh(Wh_b0),
        "wh1_f": _prep_wh(Wh_f1),
        "wh1_b": _prep_wh(Wh_b1),
    }
    wx1f = _prep_wx(Wx_f1, extra=0.5)
    wx1b = _prep_wx(Wx_b1, extra=0.5)
    wm["wx1a_f"], wm["wx1b_f"] = wx1f[:128].copy(), wx1f[128:].copy()
    wm["wx1a_b"], wm["wx1b_b"] = wx1b[:128].copy(), wx1b[128:].copy()
    return wm


def run(x, weights, T=None):
    """x: (B, T, F) fp32 full batch; weights: dict from _prep_weight_maps.
    Returns (out (B,T,2H), h_final (B,2H))."""
    from concourse.bass_utils import run_bass_kernel_spmd

    global last_exec_time_ns
    x = np.ascontiguousarray(np.asarray(x, np.float32))
    B, T_, F = x.shape
    if T is None:
        T = T_
    assert B == N_CORES * B_CORE and F == H

    nc = build_nc(T)

    in_maps = []
    for core in range(N_CORES):
        xs = x[core * B_CORE:(core + 1) * B_CORE]  # (4, T, 128)
        xT_f = np.ascontiguousarray(
            xs.transpose(2, 1, 0).reshape(128, 4 * T)).astype(np.float16)
        xT_b = np.ascontiguousarray(
            xs[:, ::-1, :].transpose(2, 1, 0).reshape(128, 4 * T)).astype(np.float16)
        im = {"xT_f": xT_f, "xT_b": xT_b}
        im.update(weights)
        in_maps.append(im)

    trace = os.environ.get("LSTM_TRACE", "0") == "1"
    if trace:
        try:
            _install_axon_prof()
        except Exception:
            pass
    res = run_bass_kernel_spmd(nc, in_maps, core_ids=list(range(N_CORES)),
                               trace=trace)
    last_exec_time_ns = res.exec_time_ns

    outs, hfs = [], []
    for core in range(N_CORES):
        r = res.results[core]
        of = r["out_f"].reshape(128, T, 4).transpose(2, 1, 0)
        ob = r["out_b"].reshape(128, T, 4)[:, ::-1, :].transpose(2, 1, 0)
        outs.append(np.concatenate([of, ob], axis=2) * 0.5)
        hf = np.concatenate([r["cfin_f"].T, r["cfin_b"].T], axis=1) * 0.5
        hfs.append(hf)
    out = np.ascontiguousarray(np.concatenate(outs, 0), np.float32)
    h_final = np.ascontiguousarray(np.concatenate(hfs, 0), np.float32)
    return out, h_final


def kernel(**inputs):
    x = np.asarray(inputs["x"], np.float32)
    for bn in ("b_f0", "b_b0", "b_f1", "b_b1"):
        bv = np.asarray(inputs[bn])
        assert np.all(bv == 0), f"nonzero bias {bn} unsupported by fast path"
    wm = _prep_weight_maps(
        inputs["Wx_f0"], inputs["Wh_f0"], inputs["Wx_b0"], inputs["Wh_b0"],
        inputs["Wx_f1"], inputs["Wh_f1"], inputs["Wx_b1"], inputs["Wh_b1"])
    return run(x, wm)


def _split_excess_waits(nc):
    """This walrus build caps sync waits at 1/instruction (2 for
    InstEventSemaphore); Tile's tail drain can exceed that. Hoist excess
    waits into standalone wait instructions (semantically identical)."""
    import concourse.mybir as mybir
    uid = [0]

    def mk(engine, wait):
        uid[0] += 1
        return mybir.InstEventSemaphore(
            name=f"WSPLIT-{uid[0]}", engine=engine, ins=[], outs=[],
            sync_info=mybir.SyncInfo(on_wait=[wait], on_update=[]))

    for f in nc.m.functions:
        for bb in f.blocks:
            out = []
            for inst in bb.instructions:
                si = inst.sync_info
                cap = 2 if isinstance(inst, mybir.InstEventSemaphore) else 1
                if si is not None and si.on_wait and len(si.on_wait) > cap:
                    excess = si.on_wait[:len(si.on_wait) - cap]
                    si.on_wait = si.on_wait[len(si.on_wait) - cap:]
                    for w_ in excess:
                        wi = mk(inst.engine, w_)
                        nc.register_instruction(wi, overwrite=True)
                        out.append(wi)
                out.append(inst)
            bb.instructions = out


def _install_axon_prof():
    """Shim antenv.axon_hooks so trace=True can NTFF-profile under axon."""
    import contextlib, ctypes, sys, types
    try:
        from antenv.axon_hooks import get_axon_ntff_profile_hook  # noqa: F401
        return
    except ImportError:
        pass
    so_path = "/opt/axon/libaxon_pjrt.so"
    lib = ctypes.CDLL(so_path)
    if not hasattr(lib, "axon_start_nrt_profile"):
        return
    lib.axon_start_nrt_profile.argtypes = [ctypes.POINTER(ctypes.c_int64), ctypes.c_size_t]
    lib.axon_start_nrt_profile.restype = ctypes.c_int64
    lib.axon_stop_nrt_profile.argtypes = [ctypes.c_char_p]
    lib.axon_stop_nrt_profile.restype = ctypes.c_int64

    @contextlib.contextmanager
    def hook(output_dir, device_ids):
        import jax
        jax.devices()
        if device_ids:
            ids = (ctypes.c_int64 * len(device_ids))(*device_ids)
            rc = lib.axon_start_nrt_profile(ids, len(device_ids))
        else:
            rc = lib.axon_start_nrt_profile(None, 0)
        if rc != 0:
            raise RuntimeError(f"axon_start_nrt_profile rc={rc}")
        try:
            yield
        finally:
            n = lib.axon_stop_nrt_profile(str(output_dir).encode())
            if n < 0:
                raise RuntimeError(f"axon_stop_nrt_profile rc={n}")

    mod = types.ModuleType("antenv.axon_hooks")
    h = [hook]
    mod.set_axon_ntff_profile_hook = lambda x: h.__setitem__(0, x)
    mod.get_axon_ntff_profile_hook = lambda: h[0]
    sys.modules["antenv.axon_hooks"] = mod
    import antenv
    antenv.axon_hooks = mod
